# revision 7
# baseline (speedup 1.0000x reference)
"""Trainium2 Bass kernel for nn_AELoss (MSE + smooth loss), 8-core data-parallel.

Strategy
--------
Shard batch dim (2048) across 8 cores -> 256 rows/core. Per core, per
(b-group of 128, c) step, DMA-load inputs+targets as ONE [128, 2, 300, 25]
bf16 tile (SWDGE cast f32->bf16 in the DMA, so HBM reads stay f32 but all
on-chip compute runs in bf16 / 2x DVE mode).

Math (per b, c, j):  with A = sum_t x[t], Q = sum_t x[t]^2:
    s_x = A - x[T-1] - Q + x[0]^2   (= sum_{t<T-1} x[t] - x[t+1]^2)
    total[b,c] = sum_{j<J-1} |s_in - s_tgt|
    smooth partial = sum_{b,c} sqrt(total) / (J*T)
    mse partial    = sum x^2 + sum y^2 - 2*sum x*y  (reuses Q sums + one
                     scalar_tensor_tensor pass with accum for the cross term)

Engines: DVE does fold-trees over t (bf16 tensor_tensor at 2x) and the
cross-term pass; ACT does the squares; gpsimd issues cast-DMAs and the final
partition reduction. Per-core partial sums are returned as a [1,2] tensor;
the host combines the 8 cores' partials into the final scalar.
"""

import os
import sys

for _p in ("/opt/trn_rl_repo", "/root/.axon_site"):
    if os.path.isdir(_p) and _p not in sys.path:
        sys.path.insert(0, _p)

import numpy as np

import concourse.bass as bass
import concourse.tile as tile
from concourse import bacc, bass_isa, mybir
from concourse.bass_utils import run_bass_kernel_spmd

N_CORES = 8
B, C, T, J = 2048, 3, 300, 25
B_LOC = B // N_CORES          # 256 batch rows per core
P = 128                       # SBUF partitions
NG = B_LOC // P               # 2 b-groups per core
F32 = mybir.dt.float32
BF16 = mybir.dt.bfloat16
NSTEP = NG * C                # 6 (b-group, c) steps


def _fold_t(nc, fs_pool, fs32_pool, src, res, lvl1_engine=None):
    """Sum src [P, 2, 300, 25] over the t axis -> res [P, 2, 25] f32.

    Binary fold tree: 300 = 2*128 + 44, then halve 128 -> 8 in bf16,
    finish 8 -> 1 in f32 (keeps the large-magnitude partials accurate).
    lvl1_engine lets the big first level run on a different engine
    (gpsimd) to offload the Vector engine.
    """
    v = nc.vector
    e1 = lvl1_engine or v
    fs = fs_pool.tile([P, 2, 128, J], BF16, tag="fold_bf")
    e1.tensor_add(fs[:, :, 0:128, :], src[:, :, 0:128, :], src[:, :, 128:256, :])
    e1.tensor_add(fs[:, :, 0:44, :], fs[:, :, 0:44, :], src[:, :, 256:300, :])
    n = 64
    while n >= 16:
        v.tensor_add(fs[:, :, 0:n, :], fs[:, :, 0:n, :], fs[:, :, n : 2 * n, :])
        n //= 2
    f32s = fs32_pool.tile([P, 2, 8, J], F32, tag="fold_f32")
    v.tensor_add(f32s[:, :, 0:8, :], fs[:, :, 0:8, :], fs[:, :, 8:16, :])
    n = 4
    while n >= 2:
        v.tensor_add(
            f32s[:, :, 0:n, :], f32s[:, :, 0:n, :], f32s[:, :, n : 2 * n, :]
        )
        n //= 2
    v.tensor_add(res[:, :, :], f32s[:, :, 0, :], f32s[:, :, 1, :])


def _body(tc, nc, x_d, y_d, out_d):
    sub = mybir.AluOpType.subtract
    add = mybir.AluOpType.add
    mult = mybir.AluOpType.mult
    bypass = mybir.AluOpType.bypass

    with (
        tc.tile_pool(name="inp", bufs=2) as inp_pool,
        tc.tile_pool(name="sq", bufs=2) as sq_pool,
        tc.tile_pool(name="fold", bufs=2) as fold_pool,
        tc.tile_pool(name="fold32", bufs=2) as fold32_pool,
        tc.tile_pool(name="small", bufs=3) as small_pool,
        tc.tile_pool(name="persist", bufs=1) as persist,
    ):
        totals6 = persist.tile([P, NSTEP], F32)       # per-step sum_j |s_in - s_tgt|
        qsum6 = persist.tile([P, 2, NSTEP], F32)      # per-step sum_{t,j} x^2 / y^2
        s2t6 = persist.tile([P, NSTEP], F32)          # per-step sum (x+y)^2

        k = 0
        for g in range(NG):
            for c in range(C):
                xy = inp_pool.tile([P, 2, T, J], BF16, tag="xy")
                nc.gpsimd.dma_start(
                    out=xy[:, 0, :, :], in_=x_d[g * P : (g + 1) * P, c, :, :]
                )
                nc.gpsimd.dma_start(
                    out=xy[:, 1, :, :], in_=y_d[g * P : (g + 1) * P, c, :, :]
                )

                # squares (ACT), both tensors in one instruction
                sq = sq_pool.tile([P, 2, T, J], BF16, tag="sq")
                nc.scalar.square(sq[:, :, :, :], xy[:, :, :, :])

                # boundary terms: e0 = x[0]^2 (f32), xl = x[T-1]
                e0 = small_pool.tile([P, 2, J], F32, tag="e0")
                nc.scalar.square(e0[:, :, :], xy[:, :, 0, :])

                # fold sums over t (A-chain level 1 offloaded to gpsimd)
                A2 = small_pool.tile([P, 2, J], F32, tag="A2")
                _fold_t(nc, fold_pool, fold32_pool, xy, A2, lvl1_engine=nc.gpsimd)
                Q2 = small_pool.tile([P, 2, J], F32, tag="Q2")
                _fold_t(nc, fold_pool, fold32_pool, sq, Q2)

                # cross term via sum (x+y)^2: s = x+y (DVE), then ACT squares
                # s in place with accumulate -> per-partition sum(x+y)^2
                s = inp_pool.tile([P, T, J], BF16, tag="s")
                nc.vector.tensor_add(s[:, :, :], xy[:, 0, :, :], xy[:, 1, :, :])
                nc.scalar.activation(
                    s[:, :, :],
                    s[:, :, :],
                    mybir.ActivationFunctionType.Square,
                    accum_out=s2t6[:, k : k + 1],
                )

                # s = A - Q + e0 - xl
                S2 = small_pool.tile([P, 2, J], F32, tag="S2")
                nc.vector.tensor_sub(S2[:, :, :], A2[:, :, :], Q2[:, :, :])
                nc.vector.tensor_add(S2[:, :, :], S2[:, :, :], e0[:, :, :])
                nc.vector.tensor_sub(S2[:, :, :], S2[:, :, :], xy[:, :, T - 1, :])

                # MSE partials: sum_j Q (all 25 j)
                nc.vector.reduce_sum(
                    qsum6[:, :, k], Q2[:, :, :], axis=mybir.AxisListType.X
                )

                # smooth: sum_{j<24} |s_in - s_tgt|
                D = small_pool.tile([P, J], F32, tag="D")
                nc.vector.tensor_sub(D[:, :], S2[:, 0, :], S2[:, 1, :])
                nc.vector.reduce_sum(
                    totals6[:, k : k + 1],
                    D[:, 0 : J - 1],
                    axis=mybir.AxisListType.X,
                    apply_absolute_value=True,
                )
                k += 1

        # tail: sqrt(total)/(J*T) == sqrt(total * (1/(J*T))^2), summed over steps
        roots = small_pool.tile([P, NSTEP], F32, tag="roots")
        nc.scalar.activation(
            roots[:, :],
            totals6[:, :],
            mybir.ActivationFunctionType.Sqrt,
            scale=1.0 / float((J * T) ** 2),
        )
        final = small_pool.tile([P, 2], F32, tag="final")
        nc.vector.reduce_sum(final[:, 1:2], roots[:, :], axis=mybir.AxisListType.X)

        qsum_red = small_pool.tile([P, 1], F32, tag="qsum_red")
        nc.vector.reduce_sum(
            qsum_red[:, :], qsum6[:, :, :], axis=mybir.AxisListType.XY
        )
        s2t_red = small_pool.tile([P, 1], F32, tag="s2t_red")
        nc.vector.reduce_sum(s2t_red[:, :], s2t6[:, :], axis=mybir.AxisListType.X)
        # mse partial: sum (x-y)^2 = 2*(sum x^2 + sum y^2) - sum (x+y)^2
        nc.vector.scalar_tensor_tensor(
            out=final[:, 0:1],
            in0=qsum_red[:, :],
            scalar=2.0,
            in1=s2t_red[:, :],
            op0=mult,
            op1=sub,
        )

        red = small_pool.tile([P, 2], F32, tag="red")
        nc.gpsimd.partition_all_reduce(
            red[:, :], final[:, :], channels=P, reduce_op=bass_isa.ReduceOp.add
        )
        nc.sync.dma_start(out=out_d[0:1, :], in_=red[0:1, :])


_NC_CACHE = None


def _build():
    global _NC_CACHE
    if _NC_CACHE is not None:
        return _NC_CACHE
    nc = bacc.Bacc("TRN2", target_bir_lowering=False, debug=False, num_devices=N_CORES)
    x_d = nc.dram_tensor("inputs", [B_LOC, C, T, J], F32, kind="ExternalInput")
    y_d = nc.dram_tensor("targets", [B_LOC, C, T, J], F32, kind="ExternalInput")
    out_d = nc.dram_tensor("out", [1, 2], F32, kind="ExternalOutput")
    with tile.TileContext(nc) as tc:
        _body(tc, nc, x_d.ap(), y_d.ap(), out_d.ap())
    nc.compile()
    _NC_CACHE = nc
    return nc


def _run(inputs, targets, trace=False, **kw):
    nc = _build()
    inputs = np.ascontiguousarray(inputs, dtype=np.float32)
    targets = np.ascontiguousarray(targets, dtype=np.float32)
    in_maps = [
        {
            "inputs": inputs[i * B_LOC : (i + 1) * B_LOC],
            "targets": targets[i * B_LOC : (i + 1) * B_LOC],
        }
        for i in range(N_CORES)
    ]
    res = run_bass_kernel_spmd(
        nc, in_maps, core_ids=list(range(N_CORES)), trace=trace, **kw
    )
    mse_sum = 0.0
    smooth_sum = 0.0
    for i in range(N_CORES):
        o = res.results[i]["out"]
        mse_sum += float(o[0, 0])
        smooth_sum += float(o[0, 1])
    value = 2.0 * (mse_sum / (B * C * T * J)) + 3.0 * (smooth_sum / (B * C))
    return np.array(value, dtype=np.float32), res


def kernel(inputs, targets):
    value, _ = _run(inputs, targets)
    return value


# revision 8
# speedup vs baseline: 1.1162x; 1.1162x over previous
"""Trainium2 Bass kernel for nn_AELoss (MSE + smooth loss), 8-core data-parallel.

Strategy
--------
Shard batch dim (2048) across 8 cores -> 256 rows/core. Per core, per
(b-group of 128, c) step, DMA-load inputs+targets as ONE [128, 2, 300, 25]
bf16 tile (SWDGE cast f32->bf16 in the DMA, so HBM reads stay f32 but all
on-chip compute runs in bf16 / 2x DVE mode).

Math (per b, c, j):  with A = sum_t x[t], Q = sum_t x[t]^2:
    s_x = A - x[T-1] - Q + x[0]^2   (= sum_{t<T-1} x[t] - x[t+1]^2)
    total[b,c] = sum_{j<J-1} |s_in - s_tgt|
    smooth partial = sum_{b,c} sqrt(total) / (J*T)
    mse partial    = sum x^2 + sum y^2 - 2*sum x*y  (reuses Q sums + one
                     scalar_tensor_tensor pass with accum for the cross term)

Engines: DVE does fold-trees over t (bf16 tensor_tensor at 2x) and the
cross-term pass; ACT does the squares; gpsimd issues cast-DMAs and the final
partition reduction. Per-core partial sums are returned as a [1,2] tensor;
the host combines the 8 cores' partials into the final scalar.
"""

import os
import sys

for _p in ("/opt/trn_rl_repo", "/root/.axon_site"):
    if os.path.isdir(_p) and _p not in sys.path:
        sys.path.insert(0, _p)

import numpy as np

import concourse.bass as bass
import concourse.tile as tile
from concourse import bacc, bass_isa, mybir
from concourse.bass_utils import run_bass_kernel_spmd

N_CORES = 8
B, C, T, J = 2048, 3, 300, 25
B_LOC = B // N_CORES          # 256 batch rows per core
P = 128                       # SBUF partitions
NG = B_LOC // P               # 2 b-groups per core
F32 = mybir.dt.float32
BF16 = mybir.dt.bfloat16
NSTEP = NG * C                # 6 (b-group, c) steps


def _fold_t(nc, fs_pool, fs32_pool, src, res, lvl1_engine=None):
    """Sum src [P, 2, 300, 25] over the t axis -> res [P, 2, 25] f32.

    Binary fold tree: 300 = 2*128 + 44, then halve 128 -> 8 in bf16,
    finish 8 -> 1 in f32 (keeps the large-magnitude partials accurate).
    lvl1_engine lets the big first level run on a different engine
    (gpsimd) to offload the Vector engine.
    """
    v = nc.vector
    e1 = lvl1_engine or v
    fs = fs_pool.tile([P, 2, 128, J], BF16, tag="fold_bf")
    e1.tensor_add(fs[:, :, 0:128, :], src[:, :, 0:128, :], src[:, :, 128:256, :])
    e1.tensor_add(fs[:, :, 0:44, :], fs[:, :, 0:44, :], src[:, :, 256:300, :])
    n = 64
    while n >= 16:
        v.tensor_add(fs[:, :, 0:n, :], fs[:, :, 0:n, :], fs[:, :, n : 2 * n, :])
        n //= 2
    f32s = fs32_pool.tile([P, 2, 8, J], F32, tag="fold_f32")
    v.tensor_add(f32s[:, :, 0:8, :], fs[:, :, 0:8, :], fs[:, :, 8:16, :])
    n = 4
    while n >= 2:
        v.tensor_add(
            f32s[:, :, 0:n, :], f32s[:, :, 0:n, :], f32s[:, :, n : 2 * n, :]
        )
        n //= 2
    v.tensor_add(res[:, :, :], f32s[:, :, 0, :], f32s[:, :, 1, :])


def _body(tc, nc, x_d, y_d, out_d):
    sub = mybir.AluOpType.subtract
    add = mybir.AluOpType.add
    mult = mybir.AluOpType.mult
    bypass = mybir.AluOpType.bypass

    with (
        tc.tile_pool(name="inp", bufs=2) as inp_pool,
        tc.tile_pool(name="sq", bufs=2) as sq_pool,
        tc.tile_pool(name="fold", bufs=2) as fold_pool,
        tc.tile_pool(name="fold32", bufs=2) as fold32_pool,
        tc.tile_pool(name="small", bufs=3) as small_pool,
        tc.tile_pool(name="persist", bufs=1) as persist,
    ):
        totals6 = persist.tile([P, NSTEP], F32)       # per-step sum_j |s_in - s_tgt|
        qsum6 = persist.tile([P, 2, NSTEP], F32)      # per-step sum_{t,j} x^2 / y^2
        s2t6 = persist.tile([P, NSTEP], F32)          # per-step sum (x+y)^2

        k = 0
        for g in range(NG):
            for c in range(C):
                xy = inp_pool.tile([P, 2, T, J], BF16, tag="xy")
                nc.gpsimd.dma_start(
                    out=xy[:, 0, :, :], in_=x_d[g * P : (g + 1) * P, c, :, :]
                )
                nc.gpsimd.dma_start(
                    out=xy[:, 1, :, :], in_=y_d[g * P : (g + 1) * P, c, :, :]
                )

                # squares (ACT), both tensors in one instruction
                sq = sq_pool.tile([P, 2, T, J], BF16, tag="sq")
                nc.scalar.square(sq[:, :, :, :], xy[:, :, :, :])

                # boundary terms: e0 = x[0]^2 (f32), xl = x[T-1]
                e0 = small_pool.tile([P, 2, J], F32, tag="e0")
                nc.scalar.square(e0[:, :, :], xy[:, :, 0, :])

                # fold sums over t (A-chain level 1 offloaded to gpsimd)
                A2 = small_pool.tile([P, 2, J], F32, tag="A2")
                _fold_t(nc, fold_pool, fold32_pool, xy, A2)
                Q2 = small_pool.tile([P, 2, J], F32, tag="Q2")
                _fold_t(nc, fold_pool, fold32_pool, sq, Q2)

                # cross term via sum (x+y)^2: s = x+y (DVE), then ACT squares
                # s in place with accumulate -> per-partition sum(x+y)^2
                s = inp_pool.tile([P, T, J], BF16, tag="s")
                nc.vector.tensor_add(s[:, :, :], xy[:, 0, :, :], xy[:, 1, :, :])
                nc.scalar.activation(
                    s[:, :, :],
                    s[:, :, :],
                    mybir.ActivationFunctionType.Square,
                    accum_out=s2t6[:, k : k + 1],
                )

                # s = A - Q + e0 - xl
                S2 = small_pool.tile([P, 2, J], F32, tag="S2")
                nc.vector.tensor_sub(S2[:, :, :], A2[:, :, :], Q2[:, :, :])
                nc.vector.tensor_add(S2[:, :, :], S2[:, :, :], e0[:, :, :])
                nc.vector.tensor_sub(S2[:, :, :], S2[:, :, :], xy[:, :, T - 1, :])

                # MSE partials: sum_j Q (all 25 j)
                nc.vector.reduce_sum(
                    qsum6[:, :, k], Q2[:, :, :], axis=mybir.AxisListType.X
                )

                # smooth: sum_{j<24} |s_in - s_tgt|
                D = small_pool.tile([P, J], F32, tag="D")
                nc.vector.tensor_sub(D[:, :], S2[:, 0, :], S2[:, 1, :])
                nc.vector.reduce_sum(
                    totals6[:, k : k + 1],
                    D[:, 0 : J - 1],
                    axis=mybir.AxisListType.X,
                    apply_absolute_value=True,
                )
                k += 1

        # tail: sqrt(total)/(J*T) == sqrt(total * (1/(J*T))^2), summed over steps
        roots = small_pool.tile([P, NSTEP], F32, tag="roots")
        nc.scalar.activation(
            roots[:, :],
            totals6[:, :],
            mybir.ActivationFunctionType.Sqrt,
            scale=1.0 / float((J * T) ** 2),
        )
        final = small_pool.tile([P, 2], F32, tag="final")
        nc.vector.reduce_sum(final[:, 1:2], roots[:, :], axis=mybir.AxisListType.X)

        qsum_red = small_pool.tile([P, 1], F32, tag="qsum_red")
        nc.vector.reduce_sum(
            qsum_red[:, :], qsum6[:, :, :], axis=mybir.AxisListType.XY
        )
        s2t_red = small_pool.tile([P, 1], F32, tag="s2t_red")
        nc.vector.reduce_sum(s2t_red[:, :], s2t6[:, :], axis=mybir.AxisListType.X)
        # mse partial: sum (x-y)^2 = 2*(sum x^2 + sum y^2) - sum (x+y)^2
        nc.vector.scalar_tensor_tensor(
            out=final[:, 0:1],
            in0=qsum_red[:, :],
            scalar=2.0,
            in1=s2t_red[:, :],
            op0=mult,
            op1=sub,
        )

        red = small_pool.tile([P, 2], F32, tag="red")
        nc.gpsimd.partition_all_reduce(
            red[:, :], final[:, :], channels=P, reduce_op=bass_isa.ReduceOp.add
        )
        nc.sync.dma_start(out=out_d[0:1, :], in_=red[0:1, :])


_NC_CACHE = None


def _build():
    global _NC_CACHE
    if _NC_CACHE is not None:
        return _NC_CACHE
    nc = bacc.Bacc("TRN2", target_bir_lowering=False, debug=False, num_devices=N_CORES)
    x_d = nc.dram_tensor("inputs", [B_LOC, C, T, J], F32, kind="ExternalInput")
    y_d = nc.dram_tensor("targets", [B_LOC, C, T, J], F32, kind="ExternalInput")
    out_d = nc.dram_tensor("out", [1, 2], F32, kind="ExternalOutput")
    with tile.TileContext(nc) as tc:
        _body(tc, nc, x_d.ap(), y_d.ap(), out_d.ap())
    nc.compile()
    _NC_CACHE = nc
    return nc


def _run(inputs, targets, trace=False, **kw):
    nc = _build()
    inputs = np.ascontiguousarray(inputs, dtype=np.float32)
    targets = np.ascontiguousarray(targets, dtype=np.float32)
    in_maps = [
        {
            "inputs": inputs[i * B_LOC : (i + 1) * B_LOC],
            "targets": targets[i * B_LOC : (i + 1) * B_LOC],
        }
        for i in range(N_CORES)
    ]
    res = run_bass_kernel_spmd(
        nc, in_maps, core_ids=list(range(N_CORES)), trace=trace, **kw
    )
    mse_sum = 0.0
    smooth_sum = 0.0
    for i in range(N_CORES):
        o = res.results[i]["out"]
        mse_sum += float(o[0, 0])
        smooth_sum += float(o[0, 1])
    value = 2.0 * (mse_sum / (B * C * T * J)) + 3.0 * (smooth_sum / (B * C))
    return np.array(value, dtype=np.float32), res


def kernel(inputs, targets):
    value, _ = _run(inputs, targets)
    return value


# revision 10
# speedup vs baseline: 1.5195x; 1.3613x over previous
"""Trainium2 Bass kernel for nn_AELoss (MSE + smooth loss), 8-core data-parallel.

Strategy
--------
Shard batch dim (2048) across 8 cores -> 256 rows/core. Per core, per
(b-group of 128, c) step, DMA-load inputs+targets as ONE [128, 2, 300, 25]
bf16 tile (SWDGE cast f32->bf16 in the DMA, so HBM reads stay f32 but all
on-chip compute runs in bf16 / 2x DVE mode).

Math (per b, c, j):  with A = sum_t x[t], Q = sum_t x[t]^2:
    s_x = A - x[T-1] - Q + x[0]^2   (= sum_{t<T-1} x[t] - x[t+1]^2)
    total[b,c] = sum_{j<J-1} |s_in - s_tgt|
    smooth partial = sum_{b,c} sqrt(total) / (J*T)
    mse partial    = sum x^2 + sum y^2 - 2*sum x*y  (reuses Q sums + one
                     scalar_tensor_tensor pass with accum for the cross term)

Engines: DVE does fold-trees over t (bf16 tensor_tensor at 2x) and the
cross-term pass; ACT does the squares; gpsimd issues cast-DMAs and the final
partition reduction. Per-core partial sums are returned as a [1,2] tensor;
the host combines the 8 cores' partials into the final scalar.
"""

import os
import sys

for _p in ("/opt/trn_rl_repo", "/root/.axon_site"):
    if os.path.isdir(_p) and _p not in sys.path:
        sys.path.insert(0, _p)

import numpy as np

import concourse.bass as bass
import concourse.tile as tile
from concourse import bacc, bass_isa, mybir
from concourse.bass_utils import run_bass_kernel_spmd

N_CORES = 8
B, C, T, J = 2048, 3, 300, 25
B_LOC = B // N_CORES          # 256 batch rows per core
P = 128                       # SBUF partitions
NG = B_LOC // P               # 2 b-groups per core
F32 = mybir.dt.float32
BF16 = mybir.dt.bfloat16
NSTEP = NG * C                # 6 (b-group, c) steps


def _fold_t(nc, fs_pool, fs32_pool, src, res):
    """Sum src [P, 300, 25] over the t axis -> res [P, 25] f32.

    Binary fold tree: 300 = 2*128 + 44, then halve 128 -> 8 in bf16,
    finish 8 -> 1 in f32 (keeps the large-magnitude partials accurate).
    """
    v = nc.vector
    fs = fs_pool.tile([P, 128, J], BF16, tag="fold_bf")
    v.tensor_add(fs[:, 0:128, :], src[:, 0:128, :], src[:, 128:256, :])
    v.tensor_add(fs[:, 0:44, :], fs[:, 0:44, :], src[:, 256:300, :])
    n = 64
    while n >= 16:
        v.tensor_add(fs[:, 0:n, :], fs[:, 0:n, :], fs[:, n : 2 * n, :])
        n //= 2
    f32s = fs32_pool.tile([P, 8, J], F32, tag="fold_f32")
    v.tensor_add(f32s[:, 0:8, :], fs[:, 0:8, :], fs[:, 8:16, :])
    n = 4
    while n >= 2:
        v.tensor_add(f32s[:, 0:n, :], f32s[:, 0:n, :], f32s[:, n : 2 * n, :])
        n //= 2
    v.tensor_add(res[:, :], f32s[:, 0, :], f32s[:, 1, :])


def _body(tc, nc, x_d, y_d, out_d):
    sub = mybir.AluOpType.subtract
    add = mybir.AluOpType.add
    mult = mybir.AluOpType.mult
    bypass = mybir.AluOpType.bypass

    with (
        tc.tile_pool(name="inp", bufs=3) as inp_pool,
        tc.tile_pool(name="sd", bufs=2) as sd_pool,
        tc.tile_pool(name="fold", bufs=4) as fold_pool,
        tc.tile_pool(name="fold32", bufs=4) as fold32_pool,
        tc.tile_pool(name="small", bufs=3) as small_pool,
        tc.tile_pool(name="persist", bufs=1) as persist,
    ):
        totals6 = persist.tile([P, NSTEP], F32)       # per-step sum_j |s_in - s_tgt|
        mse6 = persist.tile([P, NSTEP], F32)          # per-step sum (x-y)^2

        k = 0
        for g in range(NG):
            for c in range(C):
                xy = inp_pool.tile([P, 2, T, J], BF16, tag="xy")
                nc.gpsimd.dma_start(
                    out=xy[:, 0, :, :], in_=x_d[g * P : (g + 1) * P, c, :, :]
                )
                nc.gpsimd.dma_start(
                    out=xy[:, 1, :, :], in_=y_d[g * P : (g + 1) * P, c, :, :]
                )

                # s = x+y, d = x-y, p = s*d = x^2-y^2 (p overwrites s)
                s = sd_pool.tile([P, T, J], BF16, tag="s")
                nc.vector.tensor_add(s[:, :, :], xy[:, 0, :, :], xy[:, 1, :, :])
                d = sd_pool.tile([P, T, J], BF16, tag="d")
                nc.vector.tensor_sub(d[:, :, :], xy[:, 0, :, :], xy[:, 1, :, :])
                nc.vector.tensor_mul(s[:, :, :], s[:, :, :], d[:, :, :])

                # fold sums over t: Ad = sum_t d, Pd = sum_t (x^2-y^2)
                Ad = small_pool.tile([P, J], F32, tag="Ad")
                _fold_t(nc, fold_pool, fold32_pool, d, Ad)
                Pd = small_pool.tile([P, J], F32, tag="Pd")
                _fold_t(nc, fold_pool, fold32_pool, s, Pd)

                # D[j] = s_in - s_tgt = Ad - Pd + p[0] - d[T-1]
                D = small_pool.tile([P, J], F32, tag="D")
                nc.vector.tensor_sub(D[:, :], Ad[:, :], Pd[:, :])
                nc.vector.tensor_add(D[:, :], D[:, :], s[:, 0, :])
                nc.vector.tensor_sub(D[:, :], D[:, :], d[:, T - 1, :])
                nc.vector.reduce_sum(
                    totals6[:, k : k + 1],
                    D[:, 0 : J - 1],
                    axis=mybir.AxisListType.X,
                    apply_absolute_value=True,
                )

                # MSE partial: sum d^2 (ACT square in place with accumulate;
                # runs after all readers of d thanks to Tile's WAR tracking)
                nc.scalar.activation(
                    d[:, :, :],
                    d[:, :, :],
                    mybir.ActivationFunctionType.Square,
                    accum_out=mse6[:, k : k + 1],
                )
                k += 1

        # tail: sqrt(total)/(J*T) == sqrt(total * (1/(J*T))^2), summed over steps
        roots = small_pool.tile([P, NSTEP], F32, tag="roots")
        nc.scalar.activation(
            roots[:, :],
            totals6[:, :],
            mybir.ActivationFunctionType.Sqrt,
            scale=1.0 / float((J * T) ** 2),
        )
        final = small_pool.tile([P, 2], F32, tag="final")
        nc.vector.reduce_sum(final[:, 1:2], roots[:, :], axis=mybir.AxisListType.X)
        nc.vector.reduce_sum(final[:, 0:1], mse6[:, :], axis=mybir.AxisListType.X)

        red = small_pool.tile([P, 2], F32, tag="red")
        nc.gpsimd.partition_all_reduce(
            red[:, :], final[:, :], channels=P, reduce_op=bass_isa.ReduceOp.add
        )
        nc.sync.dma_start(out=out_d[0:1, :], in_=red[0:1, :])


_NC_CACHE = None


def _build():
    global _NC_CACHE
    if _NC_CACHE is not None:
        return _NC_CACHE
    nc = bacc.Bacc("TRN2", target_bir_lowering=False, debug=False, num_devices=N_CORES)
    x_d = nc.dram_tensor("inputs", [B_LOC, C, T, J], F32, kind="ExternalInput")
    y_d = nc.dram_tensor("targets", [B_LOC, C, T, J], F32, kind="ExternalInput")
    out_d = nc.dram_tensor("out", [1, 2], F32, kind="ExternalOutput")
    with tile.TileContext(nc) as tc:
        _body(tc, nc, x_d.ap(), y_d.ap(), out_d.ap())
    nc.compile()
    _NC_CACHE = nc
    return nc


def _run(inputs, targets, trace=False, **kw):
    nc = _build()
    inputs = np.ascontiguousarray(inputs, dtype=np.float32)
    targets = np.ascontiguousarray(targets, dtype=np.float32)
    in_maps = [
        {
            "inputs": inputs[i * B_LOC : (i + 1) * B_LOC],
            "targets": targets[i * B_LOC : (i + 1) * B_LOC],
        }
        for i in range(N_CORES)
    ]
    res = run_bass_kernel_spmd(
        nc, in_maps, core_ids=list(range(N_CORES)), trace=trace, **kw
    )
    mse_sum = 0.0
    smooth_sum = 0.0
    for i in range(N_CORES):
        o = res.results[i]["out"]
        mse_sum += float(o[0, 0])
        smooth_sum += float(o[0, 1])
    value = 2.0 * (mse_sum / (B * C * T * J)) + 3.0 * (smooth_sum / (B * C))
    return np.array(value, dtype=np.float32), res


def kernel(inputs, targets):
    value, _ = _run(inputs, targets)
    return value
